# revision 38
# baseline (speedup 1.0000x reference)
"""Transformer block (pre-LN, non-causal full softmax, no 1/sqrt(D) scaling)
on 8 TRN2 NeuronCores.

Sharding: core c owns batch b = c//2 and query-token half q = c%2. The host
rolls each batch's token axis so the core's query half is always rows 0:512
(full non-causal attention is permutation-invariant over key/value tokens, so
rolling the kv axis changes nothing). Each core LayerNorms + projects K/V only
for its own 512 tokens; the other half arrives from the pair core via FOUR
half-sized AllGathers over replica groups [[0,1],[2,3],[4,5],[6,7]], ordered
by when attention consumes the data (K head-pairs 0-3, V heads 0-7, K 4-7,
V 8-15) so every exchange hides behind projections or early attention pairs.

On-chip dataflow keeps activations feature-major ("h^T" = [E, tokens]) so
every matmul is lhsT.T @ rhs with the contraction on partitions. Weights are
host-cast to bf16 (Fast-Weight-Load); activations are bf16 with all
accumulation in fp32 PSUM. Attention processes head pairs: scores row-packed
(two K=64 matmuls in disjoint row groups) into one 2-bank psum tile drained
by a single wide exp, PV col-packed (two M=64 matmuls into one psum tile).
Softmax exp is split across engines: DVE computes exp via the Schraudolph
bit-trick (bf16 bits = round(A*x + B) as int16, bitcast to bf16; ~2% rel
err, fine at rel tol 2e-2) while ScalarE runs true Exp on the rest. The
softmax denominators come from two col-packed M=64 all-ones matmuls per key
chunk that accumulate the broadcast denominator directly, then
fast-reciprocal + multiply on DVE. The attn-out projection and FFN2 run
token-stationary (lhsT = activation token-chunks, weights as the moving
operand, K=1 ones-matmuls seeding the bias broadcast) so outputs land
token-major: residuals fuse at psum drain and no output transposes exist.
W2 streams as contiguous row-chunks (host layouts that need strided DMA
patterns flood the queues with tiny descriptors — measured ~32k for the
naive W2 slab rearrange).
Hard-won HW constraints: bf16 tolerates partition offsets; only one DVE
operand may read PSUM; GpSimd has no PSUM port and no tensor_scalar; fp8
q/k is numerically fatal (peaked softmax transfers top-key score error
undiluted); packed matmul pairs must be emitted adjacently; pt pool depth
beyond ~10 causes scheduler priority inversion (-53us).
"""

import ml_dtypes
import numpy as np

import concourse.bass as bass
import concourse.mybir as mybir
import concourse.tile as tile
from concourse import bacc
from concourse.bass_utils import run_bass_kernel_spmd

F32 = mybir.dt.float32
F32R = mybir.dt.float32r
BF16 = mybir.dt.bfloat16
I16 = mybir.dt.int16
FP8 = mybir.dt.float8e4
AF = mybir.ActivationFunctionType
ALU = mybir.AluOpType

B, T, E, H, D, FF = 4, 1024, 1024, 16, 64, 4096
TQ = 512
NCORES = 8
EPS = 1e-5
P = 128

# Schraudolph exp in bf16-bit space: bits = round(x * 128/ln2 + (127*128 - 6))
EXP_A = 128.0 * 1.4426950408889634
EXP_B = 127.0 * 128.0 - 6.0
# key-chunk slots whose exp runs on DVE (Schraudolph); the rest on ScalarE.
DVE_EXP_J = {5, 6, 7}

_CACHE: dict = {}


def _emit(nc, tc, d, out_d):
    const_cm = tc.tile_pool(name="const", bufs=1, side="right")
    const = const_cm.__enter__()
    eye = const.tile([P, P], BF16)
    ones_bf = const.tile([P, P], BF16)
    nc.sync.dma_start(out=eye[:], in_=d["eye"][:, :])
    nc.sync.dma_start(out=ones_bf[:], in_=d["ones_bf"][:, :])
    bprow = const.tile([1, E], BF16)
    b2row = const.tile([1, E], BF16)
    nc.sync.dma_start(out=bprow[:], in_=d["bprow"][:, :])
    nc.sync.dma_start(out=b2row[:], in_=d["b2row"][:, :])
    epst = const.tile([P, 1], F32)
    nc.vector.memset(epst[:], EPS)
    bias = {}
    for name, w in [("ln1g", 8), ("ln1b", 8), ("ln2g", 8), ("ln2b", 8),
                    ("bp", 8), ("b1", 32), ("b2", 8)]:
        bias[name] = const.tile([P, w], F32, tag=f"bias_{name}", name=f"bias_{name}")
        nc.sync.dma_start(out=bias[name][:], in_=d[name][:, :])

    # x stays resident for the stage-4 residual (no second DMA); DMAs are
    # split in column halves and high-priority so LN1 starts ASAP.
    xt_cm = tc.tile_pool(name="xres", bufs=1, side="right")
    xtp = xt_cm.__enter__()
    xt = [xtp.tile([P, E], F32, tag=f"xt{i}", name=f"xt{i}") for i in range(4)]
    engs = [nc.sync, nc.scalar, nc.gpsimd, nc.sync]
    with tc.high_priority():
        for i in range(4):
            for h2 in range(4):
                engs[h2].dma_start(
                    out=xt[i][:, h2 * 256:(h2 + 1) * 256],
                    in_=d["x"][i * P:(i + 1) * P, h2 * 256:(h2 + 1) * 256])

    # long-lived activation pools: opened just before first use (pool space
    # is reserved at open), closed right after last use; lifetimes overlap
    # non-hierarchically so they are managed manually.
    hT_cm = tc.tile_pool(name="hTp", side="left", bufs=1)
    hTp = hT_cm.__enter__()
    hT = [hTp.tile([P, TQ], BF16, tag=f"hT{j}", name=f"hT{j}") for j in range(8)]

    # ---- stage 1: LN1 + transpose to feature-major h^T ----
    # Stats on DVE, the (x-mu)*rsig normalize on ACT (as x*rsig + (-mu*rsig)),
    # psum drains split across ACT/DVE so no single engine paces the chunk.
    with tc.tile_pool(name="s1s", side="left", bufs=6) as sp, \
         tc.tile_pool(name="s1ps", bufs=4, space="PSUM") as tpp:
        for i in range(4):
            stats = sp.tile([P, 2, 6], F32, tag="stats")
            nc.vector.bn_stats(stats[:, 0, :], xt[i][:, 0:512])
            nc.vector.bn_stats(stats[:, 1, :], xt[i][:, 512:1024])
            mv = sp.tile([P, 2], F32, tag="mv")
            nc.vector.bn_aggr(mv[:], stats[:])
            rsig = sp.tile([P, 1], F32, tag="rsig")
            nc.scalar.activation(rsig[:], mv[:, 1:2], AF.Sqrt, bias=epst[:])
            nc.vector.reciprocal(rsig[:], rsig[:])
            # (x-mu)*rsig on ACT as x*rsig + (-mu*rsig): keeps the wide op
            # off DVE, which is busy with stats.
            negmu = sp.tile([P, 1], F32, tag="negmu")
            nc.vector.scalar_tensor_tensor(negmu[:], mv[:, 0:1], -1.0,
                                           rsig[:], ALU.mult, ALU.mult)
            xn = sp.tile([P, E], BF16, tag="xn")
            nc.scalar.activation(xn[:, 0:512], xt[i][:, 0:512], AF.Identity,
                                 bias=negmu[:], scale=rsig[:])
            nc.vector.tensor_scalar(xn[:, 512:1024], xt[i][:, 512:1024],
                                    mv[:, 0:1], rsig[:],
                                    ALU.subtract, op1=ALU.mult)
            for j in range(8):
                pt = tpp.tile([P, P], BF16, tag="tp")
                nc.tensor.transpose(pt[:], xn[:, j * P:(j + 1) * P], eye[:])
                if j % 2 == 0:
                    nc.scalar.activation(hT[j][:, i * P:(i + 1) * P], pt[:],
                                         AF.Identity,
                                         bias=bias["ln1b"][:, j:j + 1],
                                         scale=bias["ln1g"][:, j:j + 1])
                else:
                    nc.vector.tensor_scalar(hT[j][:, i * P:(i + 1) * P], pt[:],
                                            bias["ln1g"][:, j:j + 1],
                                            bias["ln1b"][:, j:j + 1],
                                            ALU.mult, op1=ALU.add)

    # ---- stage 2: projections K^T, V, Q^T (chunk-major storage) ----
    kv_cm = tc.tile_pool(name="kvp", bufs=1, side="right")
    kvp = kv_cm.__enter__()
    ktc = [kvp.tile([P, T], BF16, tag=f"kt{m}", name=f"kt{m}") for m in range(8)]
    qtc = [kvp.tile([P, TQ], BF16, tag=f"qt{m}", name=f"qt{m}") for m in range(8)]
    vv = [kvp.tile([P, E], BF16, tag=f"v{i}", name=f"v{i}") for i in range(8)]

    def load_w(pool, wname):
        ws = []
        for k in range(8):
            w = pool.tile([P, E], BF16, tag=f"w{k}")
            nc.sync.dma_start(out=w[:], in_=d[wname][k * P:(k + 1) * P, :])
            ws.append(w)
        return ws

    # K^T and V are computed only for this core's 512 tokens; the other
    # half arrives from the pair core via two AllGathers (K first, then V)
    # into pair-shared HBM so the exchanges overlap the V/Q projections.
    pair_groups = [[2 * g, 2 * g + 1] for g in range(4)]

    # Four half-sized AllGathers ordered by when attention needs the data:
    # K head-pairs 0-3, V heads 0-7, K head-pairs 4-7, V heads 8-15. The
    # first half of attention (pairs 0-3) only touches the first K/V halves,
    # so the later collectives hide behind attention itself.
    with tc.tile_pool(name="wk_s", side="left", bufs=1) as wp_, \
         tc.tile_pool(name="wv_s", side="left", bufs=1) as wv_, \
         tc.tile_pool(name="kl_s", side="left", bufs=4) as klp, \
         tc.tile_pool(name="ps_k", bufs=4, space="PSUM") as pp_:
        ws = load_w(wp_, "wk")
        wsv = load_w(wv_, "wv")

        def kproj_half(h):
            hx = "AB"[h]
            for mm in range(4):
                m = 4 * h + mm
                ps = pp_.tile([P, 512], F32, tag="ps")
                for k in range(8):
                    nc.tensor.matmul(ps[:], ws[k][:, m * P:(m + 1) * P],
                                     hT[k][:], start=(k == 0), stop=(k == 7))
                kl = klp.tile([P, TQ], BF16, tag="kl", name="kl")
                nc.scalar.copy(kl[:], ps[:])
                with tc.high_priority():
                    nc.sync.dma_start(out=d[f"cc_kin{hx}"][mm, :, :], in_=kl[:])
            nc.gpsimd.collective_compute(
                "AllGather", ALU.bypass, replica_groups=pair_groups,
                ins=[d[f"cc_kin{hx}"][:, :, :]],
                outs=[d[f"cc_kout{hx}"][:, :, :, :]])
            with tc.high_priority():
                for mm in range(4):
                    for sh in range(2):
                        nc.sync.dma_start(
                            out=ktc[4 * h + mm][:, sh * TQ:(sh + 1) * TQ],
                            in_=d[f"cc_kout{hx}"][sh, mm, :, :])

        def vproj_half(n):
            hx = "AB"[n]
            for i in range(4):
                ps = pp_.tile([P, 512], F32, tag="ps")
                for k in range(8):
                    nc.tensor.matmul(ps[:], hT[k][:, i * P:(i + 1) * P],
                                     wsv[k][:, n * 512:(n + 1) * 512],
                                     start=(k == 0), stop=(k == 7))
                vl = klp.tile([P, TQ], BF16, tag="vl", name="vl")
                nc.scalar.copy(vl[:], ps[:])
                with tc.high_priority():
                    nc.sync.dma_start(out=d[f"cc_vin{hx}"][i, :, :], in_=vl[:])
            nc.gpsimd.collective_compute(
                "AllGather", ALU.bypass, replica_groups=pair_groups,
                ins=[d[f"cc_vin{hx}"][:, :, :]],
                outs=[d[f"cc_vout{hx}"][:, :, :, :]])
            with tc.high_priority():
                for sh in range(2):
                    for i in range(4):
                        nc.sync.dma_start(
                            out=vv[sh * 4 + i][:, n * 512:(n + 1) * 512],
                            in_=d[f"cc_vout{hx}"][sh, i, :, :])

        kproj_half(0)
        vproj_half(0)
        kproj_half(1)
        vproj_half(1)

    with tc.tile_pool(name="wq_s", side="left", bufs=1) as wp_, \
         tc.tile_pool(name="ps_q", bufs=4, space="PSUM") as pp_:
        ws = load_w(wp_, "wq")
        for m in range(8):
            ps = pp_.tile([P, 512], F32, tag="ps")
            for k in range(8):
                nc.tensor.matmul(ps[:], ws[k][:, m * P:(m + 1) * P],
                                 hT[k][:], start=(k == 0), stop=(k == 7))
            nc.scalar.copy(qtc[m][:], ps[:])

    hT_cm.__exit__(None, None, None)

    # ---- stage 3: attention, one head-pair at a time ----
    # Pair p = heads (2p, 2p+1) living in chunk tiles ktc[p]/qtc[p] rows
    # [0:64] / [64:128]. Scores run row-packed (two K=64 matmuls in disjoint
    # row groups); exp is engine-split per DVE_EXP. PV runs col-packed (two
    # M=64 matmuls into the top/bottom halves of one psum tile) accumulating
    # chunk-major O^T, while two more col-packed M=64 all-ones matmuls
    # accumulate the denominator already broadcast over each head's 64
    # partitions. The attn-out projection follows inside the same pool
    # scope, reusing the score psum banks.
    w1g0_cm = tc.tile_pool(name="w1g0", bufs=1, side="left")
    w1g0 = w1g0_cm.__enter__()
    ws_g0 = []
    for k in range(8):
        w = w1g0.tile([P, 1024], BF16, tag=f"w1g0_{k}", name=f"w1g0_{k}")
        with tc.high_priority():
            nc.sync.dma_start(out=w[:], in_=d["w1"][k * P:(k + 1) * P, 0:1024])
        ws_g0.append(w)
    x2s_cm = tc.tile_pool(name="x2s", bufs=1, side="left")
    x2s = x2s_cm.__enter__()
    x2 = [x2s.tile([P, E], F32, tag=f"x2_{i}", name=f"x2_{i}") for i in range(4)]
    h2T = [x2s.tile([P, TQ], BF16, tag=f"h2T{j}", name=f"h2T{j}") for j in range(8)]
    ot_cm = tc.tile_pool(name="otp", side="left", bufs=1)
    otp = ot_cm.__enter__()
    otc = [otp.tile([P, TQ], BF16, tag=f"ot{p}", name=f"ot{p}") for p in range(8)]
    wps = [otp.tile([P, E], BF16, tag=f"wp{p}", name=f"wp{p}") for p in range(8)]
    with tc.tile_pool(name="att_pt", side="left", bufs=10) as ptp, \
         tc.tile_pool(name="att_sc", side="left", bufs=3) as scp, \
         tc.tile_pool(name="ps_s", bufs=2, space="PSUM") as spp, \
         tc.tile_pool(name="ps_o", bufs=2, space="PSUM") as opp, \
         tc.tile_pool(name="ps_b", bufs=2, space="PSUM") as bpp:
        state = {}

        def s_phase(p):
            ptj = []
            for j in range(8):
                # both heads' score chunks land in one 2-bank psum tile so a
                # single wide exp instruction drains them (less per-op
                # overhead on ACT/DVE).
                ps2 = spp.tile([P, 1024], F32, tag="ps_s", name="ps_s")
                for half in range(2):
                    nc.tensor.matmul(ps2[:, half * 512:half * 512 + 512],
                                     ktc[p][64 * half:64 * half + 64,
                                            j * P:(j + 1) * P],
                                     qtc[p][64 * half:64 * half + 64, :],
                                     start=True, stop=True)
                ptd = ptp.tile([P, 1024], I16, tag="pt", name="pt_")
                if j in DVE_EXP_J:
                    nc.vector.tensor_scalar(ptd[:], ps2[:], EXP_A, EXP_B,
                                            ALU.mult, op1=ALU.add)
                else:
                    nc.scalar.activation(ptd[:].bitcast(BF16), ps2[:], AF.Exp)
                ptj.append(ptd)
            state[p] = ptj

        def pv_phase(p):
            ptj = state.pop(p)
            ps_pair = opp.tile([P, 512], F32, tag="ps_o", name="ps_pair")
            ps_b = bpp.tile([P, 512], F32, tag="ps_b", name="ps_b")
            for j in range(8):
                pt0 = ptj[j][:, 0:512].bitcast(BF16)
                pt1 = ptj[j][:, 512:1024].bitcast(BF16)
                nc.tensor.matmul(ps_pair[0:64, :],
                                 vv[j][:, (2 * p) * D:(2 * p) * D + D],
                                 pt0, start=(j == 0), stop=(j == 7),
                                 skip_group_check=True)
                nc.tensor.matmul(ps_pair[64:128, :],
                                 vv[j][:, (2 * p + 1) * D:(2 * p + 1) * D + D],
                                 pt1, start=(j == 0), stop=(j == 7),
                                 skip_group_check=True)
                nc.tensor.matmul(ps_b[0:64, :], ones_bf[:, 0:64],
                                 pt0, start=(j == 0), stop=(j == 7),
                                 skip_group_check=True)
                nc.tensor.matmul(ps_b[64:128, :], ones_bf[:, 0:64],
                                 pt1, start=(j == 0), stop=(j == 7),
                                 skip_group_check=True)
            bcr = scp.tile([P, 512], F32, tag="bcr", name="bcr")
            nc.vector.reciprocal_approx_fast(bcr[:], ps_b[:])
            nc.vector.scalar_tensor_tensor(otc[p][:], ps_pair[:], 1.0,
                                           bcr[:], ALU.mult, ALU.mult)

        for p in range(8):
            nc.sync.dma_start(out=wps[p][:], in_=d["wp"][p * P:(p + 1) * P, :])
            if p >= 1:
                pv_phase(p - 1)
            s_phase(p)
        pv_phase(7)
        attn_prio = tc.cur_priority

        # attn-out projection, token-stationary: lhsT = otc token-chunks,
        # rhs = Wp row-chunks, plus a K=1 all-ones matmul seeding the bias
        # broadcast. Output lands token-major, so the residual is fused at
        # the psum drain and x2 needs no transposes at all.
        for i in range(4):
            ps2 = spp.tile([P, 1024], F32, tag="ps_s", name="psp")
            for nh in range(2):
                nc.tensor.matmul(ps2[:, nh * 512:nh * 512 + 512],
                                 ones_bf[0:1, :], bprow[:, nh * 512:nh * 512 + 512],
                                 start=True, stop=False, skip_group_check=True)
            for p in range(8):
                for nh in range(2):
                    nc.tensor.matmul(ps2[:, nh * 512:nh * 512 + 512],
                                     otc[p][:, i * P:(i + 1) * P],
                                     wps[p][:, nh * 512:nh * 512 + 512],
                                     start=False, stop=(p == 7),
                                     skip_group_check=True)
            for nh in range(2):
                nc.vector.scalar_tensor_tensor(
                    x2[i][:, nh * 512:nh * 512 + 512],
                    ps2[:, nh * 512:nh * 512 + 512], 1.0,
                    xt[i][:, nh * 512:nh * 512 + 512], ALU.mult, ALU.add)

    kv_cm.__exit__(None, None, None)

    # ---- stage 4: LN2 (x2 already holds the attn residual) ----
    with tc.tile_pool(name="s4s", side="left", bufs=6) as sp, \
         tc.tile_pool(name="s4ps", bufs=4, space="PSUM") as tpp:
        for i in range(4):
            stats = sp.tile([P, 2, 6], F32, tag="stats")
            nc.vector.bn_stats(stats[:, 0, :], x2[i][:, 0:512])
            nc.vector.bn_stats(stats[:, 1, :], x2[i][:, 512:1024])
            mv = sp.tile([P, 2], F32, tag="mv")
            nc.vector.bn_aggr(mv[:], stats[:])
            rsig = sp.tile([P, 1], F32, tag="rsig")
            nc.scalar.activation(rsig[:], mv[:, 1:2], AF.Sqrt, bias=epst[:])
            nc.vector.reciprocal(rsig[:], rsig[:])
            negmu = sp.tile([P, 1], F32, tag="negmu")
            nc.vector.scalar_tensor_tensor(negmu[:], mv[:, 0:1], -1.0,
                                           rsig[:], ALU.mult, ALU.mult)
            xn = sp.tile([P, E], BF16, tag="xn")
            nc.scalar.activation(xn[:, 0:512], x2[i][:, 0:512], AF.Identity,
                                 bias=negmu[:], scale=rsig[:])
            nc.vector.tensor_scalar(xn[:, 512:1024], x2[i][:, 512:1024],
                                    mv[:, 0:1], rsig[:],
                                    ALU.subtract, op1=ALU.mult)
            for j in range(8):
                pt = tpp.tile([P, P], BF16, tag="tp")
                nc.tensor.transpose(pt[:], xn[:, j * P:(j + 1) * P], eye[:])
                if j % 2 == 0:
                    nc.scalar.activation(h2T[j][:, i * P:(i + 1) * P], pt[:],
                                         AF.Identity,
                                         bias=bias["ln2b"][:, j:j + 1],
                                         scale=bias["ln2g"][:, j:j + 1])
                else:
                    nc.vector.tensor_scalar(h2T[j][:, i * P:(i + 1) * P], pt[:],
                                            bias["ln2g"][:, j:j + 1],
                                            bias["ln2b"][:, j:j + 1],
                                            ALU.mult, op1=ALU.add)

    ot_cm.__exit__(None, None, None)

    # ---- stage 5: FFN ----
    rr_cm = tc.tile_pool(name="relu", side="left", bufs=1)
    rrp = rr_cm.__enter__()
    rr = [rrp.tile([P, TQ], BF16, tag=f"r{k}", name=f"r{k}") for k in range(32)]

    with tc.tile_pool(name="w1s", side="left", bufs=1) as w1p, \
         tc.tile_pool(name="ps_f1", bufs=4, space="PSUM") as fpp:
        for g in range(4):
            if g == 0:
                ws = ws_g0
            else:
                ws = []
                for k in range(8):
                    w = w1p.tile([P, 1024], BF16, tag=f"w1_{k}", bufs=2)
                    nc.sync.dma_start(
                        out=w[:], in_=d["w1"][k * P:(k + 1) * P,
                                              g * 1024:(g + 1) * 1024])
                    ws.append(w)
            for m in range(8):
                ps = fpp.tile([P, TQ], F32, tag="ps")
                for k in range(8):
                    nc.tensor.matmul(ps[:], ws[k][:, m * P:(m + 1) * P],
                                     h2T[k][:], start=(k == 0), stop=(k == 7))
                col = g * 8 + m
                nc.scalar.activation(rr[col][:], ps[:], AF.Relu,
                                     bias=bias["b1"][:, col:col + 1])
    # ---- FFN2, token-stationary, fused with final residual ----
    # lhsT = rr token-chunks (stationary), rhs = raw W2 row-chunks streamed
    # once; all four token chunks accumulate in parallel across the full
    # 8-bank psum (2 banks each), seeded with a K=1 bias broadcast. Drains
    # add the x2 residual and stream straight out token-major (no
    # transposes, contiguous output DMAs).
    with tc.tile_pool(name="w2s", side="left", bufs=3) as w2p, \
         tc.tile_pool(name="outp", side="left", bufs=4) as outp, \
         tc.tile_pool(name="ps_f2", bufs=1, space="PSUM") as fpp:
        pss = []
        for i in range(4):
            ps2 = fpp.tile([P, 1024], F32, tag=f"pf{i}", name=f"pf{i}")
            for nh in range(2):
                nc.tensor.matmul(ps2[:, nh * 512:nh * 512 + 512],
                                 ones_bf[0:1, :],
                                 b2row[:, nh * 512:nh * 512 + 512],
                                 start=True, stop=False, skip_group_check=True)
            pss.append(ps2)
        for k in range(32):
            w = w2p.tile([P, E], BF16, tag="w2", name=f"w2_{k}")
            nc.sync.dma_start(out=w[:], in_=d["w2"][k * P:(k + 1) * P, :])
            for i in range(4):
                for nh in range(2):
                    nc.tensor.matmul(pss[i][:, nh * 512:nh * 512 + 512],
                                     rr[k][:, i * P:(i + 1) * P],
                                     w[:, nh * 512:nh * 512 + 512],
                                     start=False, stop=(k == 31),
                                     skip_group_check=True)
        for i in range(4):
            for nh in range(2):
                ot = outp.tile([P, TQ], F32, tag="o", name="o")
                nc.vector.scalar_tensor_tensor(
                    ot[:], pss[i][:, nh * 512:nh * 512 + 512], 1.0,
                    x2[i][:, nh * 512:nh * 512 + 512], ALU.mult, ALU.add)
                nc.sync.dma_start(
                    out=out_d[i * P:(i + 1) * P, nh * 512:nh * 512 + 512],
                    in_=ot[:])

    rr_cm.__exit__(None, None, None)
    x2s_cm.__exit__(None, None, None)
    w1g0_cm.__exit__(None, None, None)
    xt_cm.__exit__(None, None, None)
    const_cm.__exit__(None, None, None)


def _build():
    nc = bacc.Bacc("TRN2", target_bir_lowering=False, debug=False,
                num_devices=NCORES)
    d = {}

    def din(name, shape, dt=F32R):
        d[name] = nc.dram_tensor(name, shape, dt, kind="ExternalInput").ap()

    din("x", [TQ, E], F32)
    for n in ("wq", "wk", "wv"):
        din(n, [E, E], BF16)
    din("wp", [E, E], BF16)
    din("w1", [E, FF], BF16)
    din("w2", [FF, E], BF16)
    din("bprow", [1, E], BF16)
    din("b2row", [1, E], BF16)
    din("eye", [P, P], BF16)
    din("ones_bf", [P, P], BF16)
    for n, w in [("ln1g", 8), ("ln1b", 8), ("ln2g", 8), ("ln2b", 8),
                 ("bp", 8), ("b1", 32), ("b2", 8)]:
        din(n, [P, w], F32)
    out_d = nc.dram_tensor("out", [TQ, E], F32, kind="ExternalOutput").ap()
    for nm in ("cc_kinA", "cc_kinB", "cc_vinA", "cc_vinB"):
        d[nm] = nc.dram_tensor(nm, [4, P, TQ], BF16).ap()
    for nm in ("cc_koutA", "cc_koutB", "cc_voutA", "cc_voutB"):
        d[nm] = nc.dram_tensor(nm, [2, 4, P, TQ], BF16).ap()
    with nc.allow_low_precision(reason="fp32r compute"):
        with tile.TileContext(nc) as tc:
            _emit(nc, tc, d, out_d)
    nc.compile()
    return nc


def _get_nc():
    if "nc" not in _CACHE:
        _CACHE["nc"] = _build()
    return _CACHE["nc"]


def _colmajor_bias(v, width):
    return np.ascontiguousarray(np.asarray(v, np.float32).reshape(width, P).T)


def make_in_maps(x, ln1_g, ln1_b, Wq, Wk, Wv, Wp, bp, ln2_g, ln2_b,
                 W1, b1, W2, b2):
    x = np.asarray(x, dtype=np.float32)
    shared = {
        "wq": np.ascontiguousarray(
            np.transpose(np.asarray(Wq, np.float32), (1, 0, 2)).reshape(E, E)
        ).astype(ml_dtypes.bfloat16),
        "wk": np.ascontiguousarray(
            np.transpose(np.asarray(Wk, np.float32), (1, 0, 2)).reshape(E, E)
        ).astype(ml_dtypes.bfloat16),
        "wv": np.ascontiguousarray(
            np.transpose(np.asarray(Wv, np.float32), (1, 0, 2)).reshape(E, E)
        ).astype(ml_dtypes.bfloat16),
        "wp": np.asarray(Wp, np.float32).astype(ml_dtypes.bfloat16),
        "w1": np.asarray(W1, np.float32).astype(ml_dtypes.bfloat16),
        "w2": np.asarray(W2, np.float32).astype(ml_dtypes.bfloat16),
        "bprow": np.asarray(bp, np.float32).reshape(1, E).astype(
            ml_dtypes.bfloat16),
        "b2row": np.asarray(b2, np.float32).reshape(1, E).astype(
            ml_dtypes.bfloat16),
        "eye": np.eye(P, dtype=ml_dtypes.bfloat16),
        "ones_bf": np.ones((P, P), dtype=ml_dtypes.bfloat16),
        "ln1g": _colmajor_bias(ln1_g, 8),
        "ln1b": _colmajor_bias(ln1_b, 8),
        "ln2g": _colmajor_bias(ln2_g, 8),
        "ln2b": _colmajor_bias(ln2_b, 8),
        "bp": _colmajor_bias(bp, 8),
        "b1": _colmajor_bias(b1, 32),
        "b2": _colmajor_bias(b2, 8),
    }
    in_maps = []
    for c in range(NCORES):
        b = c // 2
        q0 = TQ * (c % 2)
        xb = x[b]
        x_roll = np.ascontiguousarray(np.concatenate([xb[q0:], xb[:q0]], axis=0)[:TQ])
        in_maps.append({"x": x_roll, **shared})
    return in_maps


def assemble_out(results):
    out = np.empty((B, T, E), dtype=np.float32)
    for c in range(NCORES):
        b = c // 2
        q0 = TQ * (c % 2)
        out[b, q0:q0 + TQ] = results[c]["out"]
    return out


def kernel(x, ln1_g, ln1_b, Wq, Wk, Wv, Wp, bp, ln2_g, ln2_b, W1, b1, W2, b2,
           **_ignored):
    in_maps = make_in_maps(x, ln1_g, ln1_b, Wq, Wk, Wv, Wp, bp,
                           ln2_g, ln2_b, W1, b1, W2, b2)
    nc = _get_nc()
    res = run_bass_kernel_spmd(nc, in_maps, core_ids=list(range(NCORES)))
    return assemble_out(res.results)


# revision 39
# speedup vs baseline: 1.1973x; 1.1973x over previous
"""Transformer block (pre-LN, non-causal full softmax, no 1/sqrt(D) scaling)
on 8 TRN2 NeuronCores.

Sharding: core c owns batch b = c//2 and query-token half q = c%2. The host
rolls each batch's token axis so the core's query half is always rows 0:512
(full non-causal attention is permutation-invariant over key/value tokens, so
rolling the kv axis changes nothing). Each core LayerNorms + projects K/V only
for its own 512 tokens; the other half arrives from the pair core via FOUR
half-sized AllGathers over replica groups [[0,1],[2,3],[4,5],[6,7]], ordered
by when attention consumes the data (K head-pairs 0-3, V heads 0-7, K 4-7,
V 8-15) so every exchange hides behind projections or early attention pairs.

On-chip dataflow keeps activations feature-major ("h^T" = [E, tokens]) so
every matmul is lhsT.T @ rhs with the contraction on partitions. Weights are
host-cast to bf16 (Fast-Weight-Load); activations are bf16 with all
accumulation in fp32 PSUM. Attention processes head pairs: scores row-packed
(two K=64 matmuls in disjoint row groups) into one 2-bank psum tile drained
by a single wide exp, PV col-packed (two M=64 matmuls into one psum tile).
Softmax exp is split across engines: DVE computes exp via the Schraudolph
bit-trick (bf16 bits = round(A*x + B) as int16, bitcast to bf16; ~2% rel
err, fine at rel tol 2e-2) while ScalarE runs true Exp on the rest. The
softmax denominators come from two col-packed M=64 all-ones matmuls per key
chunk that accumulate the broadcast denominator directly, then
fast-reciprocal + multiply on DVE. The attn-out projection and FFN2 run
token-stationary (lhsT = activation token-chunks, weights as the moving
operand, K=1 ones-matmuls seeding the bias broadcast) so outputs land
token-major: residuals fuse at psum drain and no output transposes exist.
W2 streams as contiguous row-chunks (host layouts that need strided DMA
patterns flood the queues with tiny descriptors — measured ~32k for the
naive W2 slab rearrange).
Hard-won HW constraints: bf16 tolerates partition offsets; only one DVE
operand may read PSUM; GpSimd has no PSUM port and no tensor_scalar; fp8
q/k is numerically fatal (peaked softmax transfers top-key score error
undiluted); packed matmul pairs must be emitted adjacently; pt pool depth
beyond ~10 causes scheduler priority inversion (-53us).
"""

import ml_dtypes
import numpy as np

import concourse.bass as bass
import concourse.mybir as mybir
import concourse.tile as tile
from concourse import bacc
from concourse.bass_utils import run_bass_kernel_spmd

F32 = mybir.dt.float32
F32R = mybir.dt.float32r
BF16 = mybir.dt.bfloat16
I16 = mybir.dt.int16
FP8 = mybir.dt.float8e4
AF = mybir.ActivationFunctionType
ALU = mybir.AluOpType

B, T, E, H, D, FF = 4, 1024, 1024, 16, 64, 4096
TQ = 512
NCORES = 8
EPS = 1e-5
P = 128

# Schraudolph exp in bf16-bit space: bits = round(x * 128/ln2 + (127*128 - 6))
EXP_A = 128.0 * 1.4426950408889634
EXP_B = 127.0 * 128.0 - 6.0
# key-chunk slots whose exp runs on DVE (Schraudolph); the rest on ScalarE.
DVE_EXP_J = {5, 6, 7}

_CACHE: dict = {}


def _emit(nc, tc, d, out_d):
    const_cm = tc.tile_pool(name="const", bufs=1, side="right")
    const = const_cm.__enter__()
    eye = const.tile([P, P], BF16)
    ones_bf = const.tile([P, P], BF16)
    nc.sync.dma_start(out=eye[:], in_=d["eye"][:, :])
    nc.sync.dma_start(out=ones_bf[:], in_=d["ones_bf"][:, :])
    bprow = const.tile([1, E], BF16)
    b2row = const.tile([1, E], BF16)
    nc.sync.dma_start(out=bprow[:], in_=d["bprow"][:, :])
    nc.sync.dma_start(out=b2row[:], in_=d["b2row"][:, :])
    epst = const.tile([P, 1], F32)
    nc.vector.memset(epst[:], EPS)
    bias = {}
    for name, w in [("ln1g", 8), ("ln1b", 8), ("ln2g", 8), ("ln2b", 8),
                    ("bp", 8), ("b1", 32), ("b2", 8)]:
        bias[name] = const.tile([P, w], F32, tag=f"bias_{name}", name=f"bias_{name}")
        nc.sync.dma_start(out=bias[name][:], in_=d[name][:, :])

    # x stays resident for the stage-4 residual (no second DMA); DMAs are
    # split in column halves and high-priority so LN1 starts ASAP.
    xt_cm = tc.tile_pool(name="xres", bufs=1, side="right")
    xtp = xt_cm.__enter__()
    xt = [xtp.tile([P, E], F32, tag=f"xt{i}", name=f"xt{i}") for i in range(4)]
    engs = [nc.sync, nc.scalar, nc.gpsimd, nc.sync]
    with tc.high_priority():
        for i in range(4):
            for h2 in range(4):
                engs[h2].dma_start(
                    out=xt[i][:, h2 * 256:(h2 + 1) * 256],
                    in_=d["x"][i * P:(i + 1) * P, h2 * 256:(h2 + 1) * 256])

    # long-lived activation pools: opened just before first use (pool space
    # is reserved at open), closed right after last use; lifetimes overlap
    # non-hierarchically so they are managed manually.
    hT_cm = tc.tile_pool(name="hTp", side="left", bufs=1)
    hTp = hT_cm.__enter__()
    hT = [hTp.tile([P, TQ], BF16, tag=f"hT{j}", name=f"hT{j}") for j in range(8)]

    # ---- stage 1: LN1 + transpose to feature-major h^T ----
    # Stats on DVE, the (x-mu)*rsig normalize on ACT (as x*rsig + (-mu*rsig)),
    # psum drains split across ACT/DVE so no single engine paces the chunk.
    with tc.tile_pool(name="s1s", side="left", bufs=6) as sp, \
         tc.tile_pool(name="s1ps", bufs=4, space="PSUM") as tpp:
        for i in range(4):
            stats = sp.tile([P, 2, 6], F32, tag="stats")
            nc.vector.bn_stats(stats[:, 0, :], xt[i][:, 0:512])
            nc.vector.bn_stats(stats[:, 1, :], xt[i][:, 512:1024])
            mv = sp.tile([P, 2], F32, tag="mv")
            nc.vector.bn_aggr(mv[:], stats[:])
            rsig = sp.tile([P, 1], F32, tag="rsig")
            nc.scalar.activation(rsig[:], mv[:, 1:2], AF.Sqrt, bias=epst[:])
            nc.vector.reciprocal(rsig[:], rsig[:])
            # (x-mu)*rsig on ACT as x*rsig + (-mu*rsig): keeps the wide op
            # off DVE, which is busy with stats.
            negmu = sp.tile([P, 1], F32, tag="negmu")
            nc.vector.scalar_tensor_tensor(negmu[:], mv[:, 0:1], -1.0,
                                           rsig[:], ALU.mult, ALU.mult)
            xn = sp.tile([P, E], BF16, tag="xn")
            nc.scalar.activation(xn[:, 0:512], xt[i][:, 0:512], AF.Identity,
                                 bias=negmu[:], scale=rsig[:])
            nc.vector.tensor_scalar(xn[:, 512:1024], xt[i][:, 512:1024],
                                    mv[:, 0:1], rsig[:],
                                    ALU.subtract, op1=ALU.mult)
            for j in range(8):
                pt = tpp.tile([P, P], BF16, tag="tp")
                nc.tensor.transpose(pt[:], xn[:, j * P:(j + 1) * P], eye[:])
                if j % 2 == 0:
                    nc.scalar.activation(hT[j][:, i * P:(i + 1) * P], pt[:],
                                         AF.Identity,
                                         bias=bias["ln1b"][:, j:j + 1],
                                         scale=bias["ln1g"][:, j:j + 1])
                else:
                    nc.vector.tensor_scalar(hT[j][:, i * P:(i + 1) * P], pt[:],
                                            bias["ln1g"][:, j:j + 1],
                                            bias["ln1b"][:, j:j + 1],
                                            ALU.mult, op1=ALU.add)

    # ---- stage 2: projections K^T, V, Q^T (chunk-major storage) ----
    kv_cm = tc.tile_pool(name="kvp", bufs=1, side="right")
    kvp = kv_cm.__enter__()
    ktc = [kvp.tile([P, T], BF16, tag=f"kt{m}", name=f"kt{m}") for m in range(8)]
    qtc = [kvp.tile([P, TQ], BF16, tag=f"qt{m}", name=f"qt{m}") for m in range(8)]
    vv = [kvp.tile([P, E], BF16, tag=f"v{i}", name=f"v{i}") for i in range(8)]

    def load_w(pool, wname):
        ws = []
        for k in range(8):
            w = pool.tile([P, E], BF16, tag=f"w{k}")
            nc.sync.dma_start(out=w[:], in_=d[wname][k * P:(k + 1) * P, :])
            ws.append(w)
        return ws

    # K^T and V are computed only for this core's 512 tokens; the other
    # half arrives from the pair core via two AllGathers (K first, then V)
    # into pair-shared HBM so the exchanges overlap the V/Q projections.
    pair_groups = [[2 * g, 2 * g + 1] for g in range(4)]

    # Four half-sized AllGathers ordered by when attention needs the data:
    # K head-pairs 0-3, V heads 0-7, K head-pairs 4-7, V heads 8-15. The
    # first half of attention (pairs 0-3) only touches the first K/V halves,
    # so the later collectives hide behind attention itself.
    with tc.tile_pool(name="wk_s", side="left", bufs=1) as wp_, \
         tc.tile_pool(name="wv_s", side="left", bufs=1) as wv_, \
         tc.tile_pool(name="kl_s", side="left", bufs=4) as klp, \
         tc.tile_pool(name="ps_k", bufs=4, space="PSUM") as pp_:
        ws = load_w(wp_, "wk")
        wsv = load_w(wv_, "wv")

        def kproj_half(h):
            hx = "AB"[h]
            for mm in range(4):
                m = 4 * h + mm
                ps = pp_.tile([P, 512], F32, tag="ps")
                for k in range(8):
                    nc.tensor.matmul(ps[:], ws[k][:, m * P:(m + 1) * P],
                                     hT[k][:], start=(k == 0), stop=(k == 7))
                kl = klp.tile([P, TQ], BF16, tag="kl", name="kl")
                nc.scalar.copy(kl[:], ps[:])
                with tc.high_priority():
                    nc.sync.dma_start(out=d[f"cc_kin{hx}"][mm, :, :], in_=kl[:])
            nc.gpsimd.collective_compute(
                "AllGather", ALU.bypass, replica_groups=pair_groups,
                ins=[d[f"cc_kin{hx}"][:, :, :]],
                outs=[d[f"cc_kout{hx}"][:, :, :, :]])
            with tc.high_priority():
                for mm in range(4):
                    for sh in range(2):
                        nc.sync.dma_start(
                            out=ktc[4 * h + mm][:, sh * TQ:(sh + 1) * TQ],
                            in_=d[f"cc_kout{hx}"][sh, mm, :, :])

        def vproj_half(n):
            hx = "AB"[n]
            for i in range(4):
                ps = pp_.tile([P, 512], F32, tag="ps")
                for k in range(8):
                    nc.tensor.matmul(ps[:], hT[k][:, i * P:(i + 1) * P],
                                     wsv[k][:, n * 512:(n + 1) * 512],
                                     start=(k == 0), stop=(k == 7))
                vl = klp.tile([P, TQ], BF16, tag="vl", name="vl")
                nc.scalar.copy(vl[:], ps[:])
                with tc.high_priority():
                    nc.sync.dma_start(out=d[f"cc_vin{hx}"][i, :, :], in_=vl[:])
            nc.gpsimd.collective_compute(
                "AllGather", ALU.bypass, replica_groups=pair_groups,
                ins=[d[f"cc_vin{hx}"][:, :, :]],
                outs=[d[f"cc_vout{hx}"][:, :, :, :]])
            with tc.high_priority():
                for sh in range(2):
                    for i in range(4):
                        nc.sync.dma_start(
                            out=vv[sh * 4 + i][:, n * 512:(n + 1) * 512],
                            in_=d[f"cc_vout{hx}"][sh, i, :, :])

        kproj_half(0)
        vproj_half(0)
        kproj_half(1)
        vproj_half(1)

    with tc.tile_pool(name="wq_s", side="left", bufs=1) as wp_, \
         tc.tile_pool(name="ps_q", bufs=4, space="PSUM") as pp_:
        ws = load_w(wp_, "wq")
        for m in range(8):
            ps = pp_.tile([P, 512], F32, tag="ps")
            for k in range(8):
                nc.tensor.matmul(ps[:], ws[k][:, m * P:(m + 1) * P],
                                 hT[k][:], start=(k == 0), stop=(k == 7))
            nc.scalar.copy(qtc[m][:], ps[:])

    hT_cm.__exit__(None, None, None)

    # ---- stage 3: attention, one head-pair at a time ----
    # Pair p = heads (2p, 2p+1) living in chunk tiles ktc[p]/qtc[p] rows
    # [0:64] / [64:128]. Scores run row-packed (two K=64 matmuls in disjoint
    # row groups); exp is engine-split per DVE_EXP. PV runs col-packed (two
    # M=64 matmuls into the top/bottom halves of one psum tile) accumulating
    # chunk-major O^T, while two more col-packed M=64 all-ones matmuls
    # accumulate the denominator already broadcast over each head's 64
    # partitions. The attn-out projection follows inside the same pool
    # scope, reusing the score psum banks.
    x2s_cm = tc.tile_pool(name="x2s", bufs=1, side="left")
    x2s = x2s_cm.__enter__()
    x2 = [x2s.tile([P, E], F32, tag=f"x2_{i}", name=f"x2_{i}") for i in range(4)]
    h2T = [x2s.tile([P, TQ], BF16, tag=f"h2T{j}", name=f"h2T{j}") for j in range(8)]
    ot_cm = tc.tile_pool(name="otp", side="left", bufs=1)
    otp = ot_cm.__enter__()
    otc = [otp.tile([P, TQ], BF16, tag=f"ot{p}", name=f"ot{p}") for p in range(8)]
    wps = [otp.tile([P, E], BF16, tag=f"wp{p}", name=f"wp{p}") for p in range(8)]
    with tc.tile_pool(name="att_pt", side="left", bufs=10) as ptp, \
         tc.tile_pool(name="att_sc", side="left", bufs=3) as scp, \
         tc.tile_pool(name="ps_s", bufs=2, space="PSUM") as spp, \
         tc.tile_pool(name="ps_o", bufs=2, space="PSUM") as opp, \
         tc.tile_pool(name="ps_b", bufs=2, space="PSUM") as bpp:
        state = {}

        def s_phase(p):
            ptj = []
            for j in range(8):
                # both heads' score chunks land in one 2-bank psum tile so a
                # single wide exp instruction drains them (less per-op
                # overhead on ACT/DVE).
                ps2 = spp.tile([P, 1024], F32, tag="ps_s", name="ps_s")
                for half in range(2):
                    nc.tensor.matmul(ps2[:, half * 512:half * 512 + 512],
                                     ktc[p][64 * half:64 * half + 64,
                                            j * P:(j + 1) * P],
                                     qtc[p][64 * half:64 * half + 64, :],
                                     start=True, stop=True)
                ptd = ptp.tile([P, 1024], I16, tag="pt", name="pt_")
                if j in DVE_EXP_J:
                    nc.vector.tensor_scalar(ptd[:], ps2[:], EXP_A, EXP_B,
                                            ALU.mult, op1=ALU.add)
                else:
                    nc.scalar.activation(ptd[:].bitcast(BF16), ps2[:], AF.Exp)
                ptj.append(ptd)
            state[p] = ptj

        def pv_phase(p):
            ptj = state.pop(p)
            ps_pair = opp.tile([P, 512], F32, tag="ps_o", name="ps_pair")
            ps_b = bpp.tile([P, 512], F32, tag="ps_b", name="ps_b")
            for j in range(8):
                pt0 = ptj[j][:, 0:512].bitcast(BF16)
                pt1 = ptj[j][:, 512:1024].bitcast(BF16)
                nc.tensor.matmul(ps_pair[0:64, :],
                                 vv[j][:, (2 * p) * D:(2 * p) * D + D],
                                 pt0, start=(j == 0), stop=(j == 7),
                                 skip_group_check=True)
                nc.tensor.matmul(ps_pair[64:128, :],
                                 vv[j][:, (2 * p + 1) * D:(2 * p + 1) * D + D],
                                 pt1, start=(j == 0), stop=(j == 7),
                                 skip_group_check=True)
                nc.tensor.matmul(ps_b[0:64, :], ones_bf[:, 0:64],
                                 pt0, start=(j == 0), stop=(j == 7),
                                 skip_group_check=True)
                nc.tensor.matmul(ps_b[64:128, :], ones_bf[:, 0:64],
                                 pt1, start=(j == 0), stop=(j == 7),
                                 skip_group_check=True)
            bcr = scp.tile([P, 512], F32, tag="bcr", name="bcr")
            nc.vector.reciprocal_approx_fast(bcr[:], ps_b[:])
            nc.vector.scalar_tensor_tensor(otc[p][:], ps_pair[:], 1.0,
                                           bcr[:], ALU.mult, ALU.mult)

        for p in range(8):
            nc.sync.dma_start(out=wps[p][:], in_=d["wp"][p * P:(p + 1) * P, :])
            if p >= 1:
                pv_phase(p - 1)
            s_phase(p)
        pv_phase(7)
        attn_prio = tc.cur_priority

        # attn-out projection, token-stationary: lhsT = otc token-chunks,
        # rhs = Wp row-chunks, plus a K=1 all-ones matmul seeding the bias
        # broadcast. Output lands token-major, so the residual is fused at
        # the psum drain and x2 needs no transposes at all.
        for i in range(4):
            ps2 = spp.tile([P, 1024], F32, tag="ps_s", name="psp")
            for nh in range(2):
                nc.tensor.matmul(ps2[:, nh * 512:nh * 512 + 512],
                                 ones_bf[0:1, :], bprow[:, nh * 512:nh * 512 + 512],
                                 start=True, stop=False, skip_group_check=True)
            for p in range(8):
                for nh in range(2):
                    nc.tensor.matmul(ps2[:, nh * 512:nh * 512 + 512],
                                     otc[p][:, i * P:(i + 1) * P],
                                     wps[p][:, nh * 512:nh * 512 + 512],
                                     start=False, stop=(p == 7),
                                     skip_group_check=True)
            for nh in range(2):
                nc.vector.scalar_tensor_tensor(
                    x2[i][:, nh * 512:nh * 512 + 512],
                    ps2[:, nh * 512:nh * 512 + 512], 1.0,
                    xt[i][:, nh * 512:nh * 512 + 512], ALU.mult, ALU.add)

    kv_cm.__exit__(None, None, None)

    # ---- stage 4: LN2 (x2 already holds the attn residual) ----
    with tc.tile_pool(name="s4s", side="left", bufs=6) as sp, \
         tc.tile_pool(name="s4ps", bufs=4, space="PSUM") as tpp:
        for i in range(4):
            stats = sp.tile([P, 2, 6], F32, tag="stats")
            nc.vector.bn_stats(stats[:, 0, :], x2[i][:, 0:512])
            nc.vector.bn_stats(stats[:, 1, :], x2[i][:, 512:1024])
            mv = sp.tile([P, 2], F32, tag="mv")
            nc.vector.bn_aggr(mv[:], stats[:])
            rsig = sp.tile([P, 1], F32, tag="rsig")
            nc.scalar.activation(rsig[:], mv[:, 1:2], AF.Sqrt, bias=epst[:])
            nc.vector.reciprocal(rsig[:], rsig[:])
            negmu = sp.tile([P, 1], F32, tag="negmu")
            nc.vector.scalar_tensor_tensor(negmu[:], mv[:, 0:1], -1.0,
                                           rsig[:], ALU.mult, ALU.mult)
            xn = sp.tile([P, E], BF16, tag="xn")
            nc.scalar.activation(xn[:, 0:512], x2[i][:, 0:512], AF.Identity,
                                 bias=negmu[:], scale=rsig[:])
            nc.vector.tensor_scalar(xn[:, 512:1024], x2[i][:, 512:1024],
                                    mv[:, 0:1], rsig[:],
                                    ALU.subtract, op1=ALU.mult)
            for j in range(8):
                pt = tpp.tile([P, P], BF16, tag="tp")
                nc.tensor.transpose(pt[:], xn[:, j * P:(j + 1) * P], eye[:])
                if j % 2 == 0:
                    nc.scalar.activation(h2T[j][:, i * P:(i + 1) * P], pt[:],
                                         AF.Identity,
                                         bias=bias["ln2b"][:, j:j + 1],
                                         scale=bias["ln2g"][:, j:j + 1])
                else:
                    nc.vector.tensor_scalar(h2T[j][:, i * P:(i + 1) * P], pt[:],
                                            bias["ln2g"][:, j:j + 1],
                                            bias["ln2b"][:, j:j + 1],
                                            ALU.mult, op1=ALU.add)

    ot_cm.__exit__(None, None, None)

    # ---- stage 5: FFN ----
    rr_cm = tc.tile_pool(name="relu", side="left", bufs=1)
    rrp = rr_cm.__enter__()
    rr = [rrp.tile([P, TQ], BF16, tag=f"r{k}", name=f"r{k}") for k in range(32)]

    with tc.tile_pool(name="w1s", side="left", bufs=1) as w1p, \
         tc.tile_pool(name="ps_f1", bufs=4, space="PSUM") as fpp:
        for g in range(4):
            ws = []
            for k in range(8):
                w = w1p.tile([P, 1024], BF16, tag=f"w1_{k}", bufs=2)
                nc.sync.dma_start(
                    out=w[:], in_=d["w1"][k * P:(k + 1) * P,
                                          g * 1024:(g + 1) * 1024])
                ws.append(w)
            for m in range(8):
                ps = fpp.tile([P, TQ], F32, tag="ps")
                for k in range(8):
                    nc.tensor.matmul(ps[:], ws[k][:, m * P:(m + 1) * P],
                                     h2T[k][:], start=(k == 0), stop=(k == 7))
                col = g * 8 + m
                nc.scalar.activation(rr[col][:], ps[:], AF.Relu,
                                     bias=bias["b1"][:, col:col + 1])
    # ---- FFN2, token-stationary, fused with final residual ----
    # lhsT = rr token-chunks (stationary), rhs = raw W2 row-chunks streamed
    # once; all four token chunks accumulate in parallel across the full
    # 8-bank psum (2 banks each), seeded with a K=1 bias broadcast. Drains
    # add the x2 residual and stream straight out token-major (no
    # transposes, contiguous output DMAs).
    with tc.tile_pool(name="w2s", side="left", bufs=3) as w2p, \
         tc.tile_pool(name="outp", side="left", bufs=4) as outp, \
         tc.tile_pool(name="ps_f2", bufs=1, space="PSUM") as fpp:
        pss = []
        for i in range(4):
            ps2 = fpp.tile([P, 1024], F32, tag=f"pf{i}", name=f"pf{i}")
            for nh in range(2):
                nc.tensor.matmul(ps2[:, nh * 512:nh * 512 + 512],
                                 ones_bf[0:1, :],
                                 b2row[:, nh * 512:nh * 512 + 512],
                                 start=True, stop=False, skip_group_check=True)
            pss.append(ps2)
        for k in range(32):
            w = w2p.tile([P, E], BF16, tag="w2", name=f"w2_{k}")
            nc.sync.dma_start(out=w[:], in_=d["w2"][k * P:(k + 1) * P, :])
            for i in range(4):
                for nh in range(2):
                    nc.tensor.matmul(pss[i][:, nh * 512:nh * 512 + 512],
                                     rr[k][:, i * P:(i + 1) * P],
                                     w[:, nh * 512:nh * 512 + 512],
                                     start=False, stop=(k == 31),
                                     skip_group_check=True)
        for i in range(4):
            for nh in range(2):
                ot = outp.tile([P, TQ], F32, tag="o", name="o")
                nc.vector.scalar_tensor_tensor(
                    ot[:], pss[i][:, nh * 512:nh * 512 + 512], 1.0,
                    x2[i][:, nh * 512:nh * 512 + 512], ALU.mult, ALU.add)
                nc.sync.dma_start(
                    out=out_d[i * P:(i + 1) * P, nh * 512:nh * 512 + 512],
                    in_=ot[:])

    rr_cm.__exit__(None, None, None)
    x2s_cm.__exit__(None, None, None)
    xt_cm.__exit__(None, None, None)
    const_cm.__exit__(None, None, None)


def _build():
    nc = bacc.Bacc("TRN2", target_bir_lowering=False, debug=False,
                num_devices=NCORES)
    d = {}

    def din(name, shape, dt=F32R):
        d[name] = nc.dram_tensor(name, shape, dt, kind="ExternalInput").ap()

    din("x", [TQ, E], F32)
    for n in ("wq", "wk", "wv"):
        din(n, [E, E], BF16)
    din("wp", [E, E], BF16)
    din("w1", [E, FF], BF16)
    din("w2", [FF, E], BF16)
    din("bprow", [1, E], BF16)
    din("b2row", [1, E], BF16)
    din("eye", [P, P], BF16)
    din("ones_bf", [P, P], BF16)
    for n, w in [("ln1g", 8), ("ln1b", 8), ("ln2g", 8), ("ln2b", 8),
                 ("bp", 8), ("b1", 32), ("b2", 8)]:
        din(n, [P, w], F32)
    out_d = nc.dram_tensor("out", [TQ, E], F32, kind="ExternalOutput").ap()
    for nm in ("cc_kinA", "cc_kinB", "cc_vinA", "cc_vinB"):
        d[nm] = nc.dram_tensor(nm, [4, P, TQ], BF16).ap()
    for nm in ("cc_koutA", "cc_koutB", "cc_voutA", "cc_voutB"):
        d[nm] = nc.dram_tensor(nm, [2, 4, P, TQ], BF16).ap()
    with nc.allow_low_precision(reason="fp32r compute"):
        with tile.TileContext(nc) as tc:
            _emit(nc, tc, d, out_d)
    nc.compile()
    return nc


def _get_nc():
    if "nc" not in _CACHE:
        _CACHE["nc"] = _build()
    return _CACHE["nc"]


def _colmajor_bias(v, width):
    return np.ascontiguousarray(np.asarray(v, np.float32).reshape(width, P).T)


def make_in_maps(x, ln1_g, ln1_b, Wq, Wk, Wv, Wp, bp, ln2_g, ln2_b,
                 W1, b1, W2, b2):
    x = np.asarray(x, dtype=np.float32)
    shared = {
        "wq": np.ascontiguousarray(
            np.transpose(np.asarray(Wq, np.float32), (1, 0, 2)).reshape(E, E)
        ).astype(ml_dtypes.bfloat16),
        "wk": np.ascontiguousarray(
            np.transpose(np.asarray(Wk, np.float32), (1, 0, 2)).reshape(E, E)
        ).astype(ml_dtypes.bfloat16),
        "wv": np.ascontiguousarray(
            np.transpose(np.asarray(Wv, np.float32), (1, 0, 2)).reshape(E, E)
        ).astype(ml_dtypes.bfloat16),
        "wp": np.asarray(Wp, np.float32).astype(ml_dtypes.bfloat16),
        "w1": np.asarray(W1, np.float32).astype(ml_dtypes.bfloat16),
        "w2": np.asarray(W2, np.float32).astype(ml_dtypes.bfloat16),
        "bprow": np.asarray(bp, np.float32).reshape(1, E).astype(
            ml_dtypes.bfloat16),
        "b2row": np.asarray(b2, np.float32).reshape(1, E).astype(
            ml_dtypes.bfloat16),
        "eye": np.eye(P, dtype=ml_dtypes.bfloat16),
        "ones_bf": np.ones((P, P), dtype=ml_dtypes.bfloat16),
        "ln1g": _colmajor_bias(ln1_g, 8),
        "ln1b": _colmajor_bias(ln1_b, 8),
        "ln2g": _colmajor_bias(ln2_g, 8),
        "ln2b": _colmajor_bias(ln2_b, 8),
        "bp": _colmajor_bias(bp, 8),
        "b1": _colmajor_bias(b1, 32),
        "b2": _colmajor_bias(b2, 8),
    }
    in_maps = []
    for c in range(NCORES):
        b = c // 2
        q0 = TQ * (c % 2)
        xb = x[b]
        x_roll = np.ascontiguousarray(np.concatenate([xb[q0:], xb[:q0]], axis=0)[:TQ])
        in_maps.append({"x": x_roll, **shared})
    return in_maps


def assemble_out(results):
    out = np.empty((B, T, E), dtype=np.float32)
    for c in range(NCORES):
        b = c // 2
        q0 = TQ * (c % 2)
        out[b, q0:q0 + TQ] = results[c]["out"]
    return out


def kernel(x, ln1_g, ln1_b, Wq, Wk, Wv, Wp, bp, ln2_g, ln2_b, W1, b1, W2, b2,
           **_ignored):
    in_maps = make_in_maps(x, ln1_g, ln1_b, Wq, Wk, Wv, Wp, bp,
                           ln2_g, ln2_b, W1, b1, W2, b2)
    nc = _get_nc()
    res = run_bass_kernel_spmd(nc, in_maps, core_ids=list(range(NCORES)))
    return assemble_out(res.results)


# revision 41
# speedup vs baseline: 1.2003x; 1.0025x over previous
"""Transformer block (pre-LN, non-causal full softmax, no 1/sqrt(D) scaling)
on 8 TRN2 NeuronCores.

Sharding: core c owns batch b = c//2 and query-token half q = c%2. The host
rolls each batch's token axis so the core's query half is always rows 0:512
(full non-causal attention is permutation-invariant over key/value tokens, so
rolling the kv axis changes nothing). Each core LayerNorms + projects K/V only
for its own 512 tokens; the other half arrives from the pair core via FOUR
half-sized AllGathers over replica groups [[0,1],[2,3],[4,5],[6,7]], ordered
by when attention consumes the data (K head-pairs 0-3, V heads 0-7, K 4-7,
V 8-15) so every exchange hides behind projections or early attention pairs.

On-chip dataflow keeps activations feature-major ("h^T" = [E, tokens]) so
every matmul is lhsT.T @ rhs with the contraction on partitions. Weights are
host-cast to bf16 (Fast-Weight-Load); activations are bf16 with all
accumulation in fp32 PSUM. Attention processes head pairs: scores row-packed
(two K=64 matmuls in disjoint row groups) into one 2-bank psum tile drained
by a single wide exp, PV col-packed (two M=64 matmuls into one psum tile).
Softmax exp is split across engines: DVE computes exp via the Schraudolph
bit-trick (bf16 bits = round(A*x + B) as int16, bitcast to bf16; ~2% rel
err, fine at rel tol 2e-2) while ScalarE runs true Exp on the rest. The
softmax denominators come from two col-packed M=64 all-ones matmuls per key
chunk that accumulate the broadcast denominator directly, then
fast-reciprocal + multiply on DVE. The attn-out projection and FFN2 run
token-stationary (lhsT = activation token-chunks, weights as the moving
operand, K=1 ones-matmuls seeding the bias broadcast) so outputs land
token-major: residuals fuse at psum drain and no output transposes exist.
W2 streams as contiguous row-chunks (host layouts that need strided DMA
patterns flood the queues with tiny descriptors — measured ~32k for the
naive W2 slab rearrange).
Hard-won HW constraints: bf16 tolerates partition offsets; only one DVE
operand may read PSUM; GpSimd has no PSUM port and no tensor_scalar; fp8
q/k is numerically fatal (peaked softmax transfers top-key score error
undiluted); packed matmul pairs must be emitted adjacently; pt pool depth
beyond ~10 causes scheduler priority inversion (-53us).
"""

import ml_dtypes
import numpy as np

import concourse.bass as bass
import concourse.mybir as mybir
import concourse.tile as tile
from concourse import bacc
from concourse.bass_utils import run_bass_kernel_spmd

F32 = mybir.dt.float32
F32R = mybir.dt.float32r
BF16 = mybir.dt.bfloat16
I16 = mybir.dt.int16
FP8 = mybir.dt.float8e4
AF = mybir.ActivationFunctionType
ALU = mybir.AluOpType

B, T, E, H, D, FF = 4, 1024, 1024, 16, 64, 4096
TQ = 512
NCORES = 8
EPS = 1e-5
P = 128

# Schraudolph exp in bf16-bit space: bits = round(x * 128/ln2 + (127*128 - 6))
EXP_A = 128.0 * 1.4426950408889634
EXP_B = 127.0 * 128.0 - 6.0
# key-chunk slots whose exp runs on DVE (Schraudolph); the rest on ScalarE.
DVE_EXP_J = {5, 6, 7}

_CACHE: dict = {}


def _emit(nc, tc, d, out_d):
    const_cm = tc.tile_pool(name="const", bufs=1, side="right")
    const = const_cm.__enter__()
    eye = const.tile([P, P], BF16)
    ones_bf = const.tile([P, P], BF16)
    nc.sync.dma_start(out=eye[:], in_=d["eye"][:, :])
    nc.sync.dma_start(out=ones_bf[:], in_=d["ones_bf"][:, :])
    bprow = const.tile([1, E], BF16)
    b2row = const.tile([1, E], BF16)
    nc.sync.dma_start(out=bprow[:], in_=d["bprow"][:, :])
    nc.sync.dma_start(out=b2row[:], in_=d["b2row"][:, :])
    epst = const.tile([P, 1], F32)
    nc.vector.memset(epst[:], EPS)
    bias = {}
    for name, w in [("ln1g", 8), ("ln1b", 8), ("ln2g", 8), ("ln2b", 8),
                    ("bp", 8), ("b1", 32), ("b2", 8)]:
        bias[name] = const.tile([P, w], F32, tag=f"bias_{name}", name=f"bias_{name}")
        nc.sync.dma_start(out=bias[name][:], in_=d[name][:, :])

    # x stays resident for the stage-4 residual (no second DMA); DMAs are
    # split in column halves and high-priority so LN1 starts ASAP.
    xt_cm = tc.tile_pool(name="xres", bufs=1, side="right")
    xtp = xt_cm.__enter__()
    xt = [xtp.tile([P, E], F32, tag=f"xt{i}", name=f"xt{i}") for i in range(4)]
    engs = [nc.sync, nc.scalar, nc.gpsimd, nc.sync]
    with tc.high_priority():
        for i in range(4):
            for h2 in range(4):
                engs[h2].dma_start(
                    out=xt[i][:, h2 * 256:(h2 + 1) * 256],
                    in_=d["x"][i * P:(i + 1) * P, h2 * 256:(h2 + 1) * 256])

    # long-lived activation pools: opened just before first use (pool space
    # is reserved at open), closed right after last use; lifetimes overlap
    # non-hierarchically so they are managed manually.
    hT_cm = tc.tile_pool(name="hTp", side="left", bufs=1)
    hTp = hT_cm.__enter__()
    hT = [hTp.tile([P, TQ], BF16, tag=f"hT{j}", name=f"hT{j}") for j in range(8)]

    # ---- stage 1: LN1 + transpose to feature-major h^T ----
    # Stats on DVE, the (x-mu)*rsig normalize on ACT (as x*rsig + (-mu*rsig)),
    # psum drains split across ACT/DVE so no single engine paces the chunk.
    with tc.tile_pool(name="s1s", side="left", bufs=6) as sp, \
         tc.tile_pool(name="s1ps", bufs=4, space="PSUM") as tpp:
        for i in range(4):
            stats = sp.tile([P, 2, 6], F32, tag="stats")
            nc.vector.bn_stats(stats[:, 0, :], xt[i][:, 0:512])
            nc.vector.bn_stats(stats[:, 1, :], xt[i][:, 512:1024])
            mv = sp.tile([P, 2], F32, tag="mv")
            nc.vector.bn_aggr(mv[:], stats[:])
            rsig = sp.tile([P, 1], F32, tag="rsig")
            nc.scalar.activation(rsig[:], mv[:, 1:2], AF.Sqrt, bias=epst[:])
            nc.vector.reciprocal(rsig[:], rsig[:])
            # (x-mu)*rsig on ACT as x*rsig + (-mu*rsig): keeps the wide op
            # off DVE, which is busy with stats.
            negmu = sp.tile([P, 1], F32, tag="negmu")
            nc.vector.scalar_tensor_tensor(negmu[:], mv[:, 0:1], -1.0,
                                           rsig[:], ALU.mult, ALU.mult)
            xn = sp.tile([P, E], BF16, tag="xn")
            nc.scalar.activation(xn[:, 0:512], xt[i][:, 0:512], AF.Identity,
                                 bias=negmu[:], scale=rsig[:])
            nc.vector.tensor_scalar(xn[:, 512:1024], xt[i][:, 512:1024],
                                    mv[:, 0:1], rsig[:],
                                    ALU.subtract, op1=ALU.mult)
            for j in range(8):
                pt = tpp.tile([P, P], BF16, tag="tp")
                nc.tensor.transpose(pt[:], xn[:, j * P:(j + 1) * P], eye[:])
                if j % 2 == 0:
                    nc.scalar.activation(hT[j][:, i * P:(i + 1) * P], pt[:],
                                         AF.Identity,
                                         bias=bias["ln1b"][:, j:j + 1],
                                         scale=bias["ln1g"][:, j:j + 1])
                else:
                    nc.vector.tensor_scalar(hT[j][:, i * P:(i + 1) * P], pt[:],
                                            bias["ln1g"][:, j:j + 1],
                                            bias["ln1b"][:, j:j + 1],
                                            ALU.mult, op1=ALU.add)

    # ---- stage 2: projections K^T, V, Q^T (chunk-major storage) ----
    kv_cm = tc.tile_pool(name="kvp", bufs=1, side="right")
    kvp = kv_cm.__enter__()
    ktc = [kvp.tile([P, T], BF16, tag=f"kt{m}", name=f"kt{m}") for m in range(8)]
    qtc = [kvp.tile([P, TQ], BF16, tag=f"qt{m}", name=f"qt{m}") for m in range(8)]
    vv = [kvp.tile([P, E], BF16, tag=f"v{i}", name=f"v{i}") for i in range(8)]

    def load_w(pool, wname):
        ws = []
        for k in range(8):
            w = pool.tile([P, E], BF16, tag=f"w{k}")
            nc.sync.dma_start(out=w[:], in_=d[wname][k * P:(k + 1) * P, :])
            ws.append(w)
        return ws

    # K^T and V are computed only for this core's 512 tokens; the other
    # half arrives from the pair core via two AllGathers (K first, then V)
    # into pair-shared HBM so the exchanges overlap the V/Q projections.
    pair_groups = [[2 * g, 2 * g + 1] for g in range(4)]

    # Four half-sized AllGathers ordered by when attention needs the data:
    # K head-pairs 0-3, V heads 0-7, K head-pairs 4-7, V heads 8-15. The
    # first half of attention (pairs 0-3) only touches the first K/V halves,
    # so the later collectives hide behind attention itself.
    with tc.tile_pool(name="wk_s", side="left", bufs=1) as wp_, \
         tc.tile_pool(name="wv_s", side="left", bufs=1) as wv_, \
         tc.tile_pool(name="kl_s", side="left", bufs=4) as klp, \
         tc.tile_pool(name="ps_k", bufs=4, space="PSUM") as pp_:
        ws = load_w(wp_, "wk")
        wsv = load_w(wv_, "wv")

        def kproj_half(h):
            hx = "AB"[h]
            for mm in range(4):
                m = 4 * h + mm
                ps = pp_.tile([P, 512], F32, tag="ps")
                for k in range(8):
                    nc.tensor.matmul(ps[:], ws[k][:, m * P:(m + 1) * P],
                                     hT[k][:], start=(k == 0), stop=(k == 7))
                kl = klp.tile([P, TQ], BF16, tag="kl", name="kl")
                nc.scalar.copy(kl[:], ps[:])
                with tc.high_priority():
                    nc.sync.dma_start(out=d[f"cc_kin{hx}"][mm, :, :], in_=kl[:])
            nc.gpsimd.collective_compute(
                "AllGather", ALU.bypass, replica_groups=pair_groups,
                ins=[d[f"cc_kin{hx}"][:, :, :]],
                outs=[d[f"cc_kout{hx}"][:, :, :, :]])
            with tc.high_priority():
                for mm in range(4):
                    for sh in range(2):
                        nc.sync.dma_start(
                            out=ktc[4 * h + mm][:, sh * TQ:(sh + 1) * TQ],
                            in_=d[f"cc_kout{hx}"][sh, mm, :, :])

        def vproj_half(n):
            hx = "AB"[n]
            for i in range(4):
                ps = pp_.tile([P, 512], F32, tag="ps")
                for k in range(8):
                    nc.tensor.matmul(ps[:], hT[k][:, i * P:(i + 1) * P],
                                     wsv[k][:, n * 512:(n + 1) * 512],
                                     start=(k == 0), stop=(k == 7))
                vl = klp.tile([P, TQ], BF16, tag="vl", name="vl")
                nc.scalar.copy(vl[:], ps[:])
                with tc.high_priority():
                    nc.sync.dma_start(out=d[f"cc_vin{hx}"][i, :, :], in_=vl[:])
            nc.gpsimd.collective_compute(
                "AllGather", ALU.bypass, replica_groups=pair_groups,
                ins=[d[f"cc_vin{hx}"][:, :, :]],
                outs=[d[f"cc_vout{hx}"][:, :, :, :]])
            with tc.high_priority():
                for sh in range(2):
                    for i in range(4):
                        nc.sync.dma_start(
                            out=vv[sh * 4 + i][:, n * 512:(n + 1) * 512],
                            in_=d[f"cc_vout{hx}"][sh, i, :, :])

        kproj_half(0)
        vproj_half(0)
        kproj_half(1)
        vproj_half(1)

    with tc.tile_pool(name="wq_s", side="left", bufs=1) as wp_, \
         tc.tile_pool(name="ps_q", bufs=4, space="PSUM") as pp_:
        ws = load_w(wp_, "wq")
        for m in range(8):
            ps = pp_.tile([P, 512], F32, tag="ps")
            for k in range(8):
                nc.tensor.matmul(ps[:], ws[k][:, m * P:(m + 1) * P],
                                 hT[k][:], start=(k == 0), stop=(k == 7))
            nc.scalar.copy(qtc[m][:], ps[:])

    hT_cm.__exit__(None, None, None)

    # ---- stage 3: attention, one head-pair at a time ----
    # Pair p = heads (2p, 2p+1) living in chunk tiles ktc[p]/qtc[p] rows
    # [0:64] / [64:128]. Scores run row-packed (two K=64 matmuls in disjoint
    # row groups); exp is engine-split per DVE_EXP. PV runs col-packed (two
    # M=64 matmuls into the top/bottom halves of one psum tile) accumulating
    # chunk-major O^T, while two more col-packed M=64 all-ones matmuls
    # accumulate the denominator already broadcast over each head's 64
    # partitions. The attn-out projection follows inside the same pool
    # scope, reusing the score psum banks.
    w1g0_cm = tc.tile_pool(name="w1g0", bufs=1, side="left")
    w1g0 = w1g0_cm.__enter__()
    ws_g0 = [w1g0.tile([P, 1024], BF16, tag=f"w1g0_{k}", name=f"w1g0_{k}")
             for k in range(8)]
    x2s_cm = tc.tile_pool(name="x2s", bufs=1, side="left")
    x2s = x2s_cm.__enter__()
    x2 = [x2s.tile([P, E], F32, tag=f"x2_{i}", name=f"x2_{i}") for i in range(4)]
    h2T = [x2s.tile([P, TQ], BF16, tag=f"h2T{j}", name=f"h2T{j}") for j in range(8)]
    ot_cm = tc.tile_pool(name="otp", side="left", bufs=1)
    otp = ot_cm.__enter__()
    otc = [otp.tile([P, TQ], BF16, tag=f"ot{p}", name=f"ot{p}") for p in range(8)]
    wps = [otp.tile([P, E], BF16, tag=f"wp{p}", name=f"wp{p}") for p in range(8)]
    with tc.tile_pool(name="att_pt", side="left", bufs=10) as ptp, \
         tc.tile_pool(name="att_sc", side="left", bufs=3) as scp, \
         tc.tile_pool(name="ps_s", bufs=2, space="PSUM") as spp, \
         tc.tile_pool(name="ps_o", bufs=2, space="PSUM") as opp, \
         tc.tile_pool(name="ps_b", bufs=2, space="PSUM") as bpp:
        state = {}

        def s_phase(p):
            ptj = []
            for j in range(8):
                # both heads' score chunks land in one 2-bank psum tile so a
                # single wide exp instruction drains them (less per-op
                # overhead on ACT/DVE).
                ps2 = spp.tile([P, 1024], F32, tag="ps_s", name="ps_s")
                for half in range(2):
                    nc.tensor.matmul(ps2[:, half * 512:half * 512 + 512],
                                     ktc[p][64 * half:64 * half + 64,
                                            j * P:(j + 1) * P],
                                     qtc[p][64 * half:64 * half + 64, :],
                                     start=True, stop=True)
                ptd = ptp.tile([P, 1024], I16, tag="pt", name="pt_")
                if j in DVE_EXP_J:
                    nc.vector.tensor_scalar(ptd[:], ps2[:], EXP_A, EXP_B,
                                            ALU.mult, op1=ALU.add)
                else:
                    nc.scalar.activation(ptd[:].bitcast(BF16), ps2[:], AF.Exp)
                ptj.append(ptd)
            state[p] = ptj

        def pv_phase(p):
            ptj = state.pop(p)
            ps_pair = opp.tile([P, 512], F32, tag="ps_o", name="ps_pair")
            ps_b = bpp.tile([P, 512], F32, tag="ps_b", name="ps_b")
            for j in range(8):
                pt0 = ptj[j][:, 0:512].bitcast(BF16)
                pt1 = ptj[j][:, 512:1024].bitcast(BF16)
                nc.tensor.matmul(ps_pair[0:64, :],
                                 vv[j][:, (2 * p) * D:(2 * p) * D + D],
                                 pt0, start=(j == 0), stop=(j == 7),
                                 skip_group_check=True)
                nc.tensor.matmul(ps_pair[64:128, :],
                                 vv[j][:, (2 * p + 1) * D:(2 * p + 1) * D + D],
                                 pt1, start=(j == 0), stop=(j == 7),
                                 skip_group_check=True)
                nc.tensor.matmul(ps_b[0:64, :], ones_bf[:, 0:64],
                                 pt0, start=(j == 0), stop=(j == 7),
                                 skip_group_check=True)
                nc.tensor.matmul(ps_b[64:128, :], ones_bf[:, 0:64],
                                 pt1, start=(j == 0), stop=(j == 7),
                                 skip_group_check=True)
            bcr = scp.tile([P, 512], F32, tag="bcr", name="bcr")
            nc.vector.reciprocal_approx_fast(bcr[:], ps_b[:])
            nc.vector.scalar_tensor_tensor(otc[p][:], ps_pair[:], 1.0,
                                           bcr[:], ALU.mult, ALU.mult)

        for p in range(8):
            nc.sync.dma_start(out=wps[p][:], in_=d["wp"][p * P:(p + 1) * P, :])
            if 3 <= p <= 6:
                for k in (2 * (p - 3), 2 * (p - 3) + 1):
                    nc.sync.dma_start(
                        out=ws_g0[k][:], in_=d["w1"][k * P:(k + 1) * P, 0:1024])
            if p >= 1:
                pv_phase(p - 1)
            s_phase(p)
        pv_phase(7)

        # attn-out projection, token-stationary: lhsT = otc token-chunks,
        # rhs = Wp row-chunks, plus a K=1 all-ones matmul seeding the bias
        # broadcast. Output lands token-major, so the residual is fused at
        # the psum drain and x2 needs no transposes at all.
        for i in range(4):
            ps2 = spp.tile([P, 1024], F32, tag="ps_s", name="psp")
            for nh in range(2):
                nc.tensor.matmul(ps2[:, nh * 512:nh * 512 + 512],
                                 ones_bf[0:1, :], bprow[:, nh * 512:nh * 512 + 512],
                                 start=True, stop=False, skip_group_check=True)
            for p in range(8):
                for nh in range(2):
                    nc.tensor.matmul(ps2[:, nh * 512:nh * 512 + 512],
                                     otc[p][:, i * P:(i + 1) * P],
                                     wps[p][:, nh * 512:nh * 512 + 512],
                                     start=False, stop=(p == 7),
                                     skip_group_check=True)
            for nh in range(2):
                nc.vector.scalar_tensor_tensor(
                    x2[i][:, nh * 512:nh * 512 + 512],
                    ps2[:, nh * 512:nh * 512 + 512], 1.0,
                    xt[i][:, nh * 512:nh * 512 + 512], ALU.mult, ALU.add)

    kv_cm.__exit__(None, None, None)

    # ---- stage 4: LN2 (x2 already holds the attn residual) ----
    with tc.tile_pool(name="s4s", side="left", bufs=6) as sp, \
         tc.tile_pool(name="s4ps", bufs=4, space="PSUM") as tpp:
        for i in range(4):
            stats = sp.tile([P, 2, 6], F32, tag="stats")
            nc.vector.bn_stats(stats[:, 0, :], x2[i][:, 0:512])
            nc.vector.bn_stats(stats[:, 1, :], x2[i][:, 512:1024])
            mv = sp.tile([P, 2], F32, tag="mv")
            nc.vector.bn_aggr(mv[:], stats[:])
            rsig = sp.tile([P, 1], F32, tag="rsig")
            nc.scalar.activation(rsig[:], mv[:, 1:2], AF.Sqrt, bias=epst[:])
            nc.vector.reciprocal(rsig[:], rsig[:])
            negmu = sp.tile([P, 1], F32, tag="negmu")
            nc.vector.scalar_tensor_tensor(negmu[:], mv[:, 0:1], -1.0,
                                           rsig[:], ALU.mult, ALU.mult)
            xn = sp.tile([P, E], BF16, tag="xn")
            nc.scalar.activation(xn[:, 0:512], x2[i][:, 0:512], AF.Identity,
                                 bias=negmu[:], scale=rsig[:])
            nc.vector.tensor_scalar(xn[:, 512:1024], x2[i][:, 512:1024],
                                    mv[:, 0:1], rsig[:],
                                    ALU.subtract, op1=ALU.mult)
            for j in range(8):
                pt = tpp.tile([P, P], BF16, tag="tp")
                nc.tensor.transpose(pt[:], xn[:, j * P:(j + 1) * P], eye[:])
                if j % 2 == 0:
                    nc.scalar.activation(h2T[j][:, i * P:(i + 1) * P], pt[:],
                                         AF.Identity,
                                         bias=bias["ln2b"][:, j:j + 1],
                                         scale=bias["ln2g"][:, j:j + 1])
                else:
                    nc.vector.tensor_scalar(h2T[j][:, i * P:(i + 1) * P], pt[:],
                                            bias["ln2g"][:, j:j + 1],
                                            bias["ln2b"][:, j:j + 1],
                                            ALU.mult, op1=ALU.add)

    ot_cm.__exit__(None, None, None)

    # ---- stage 5: FFN ----
    rr_cm = tc.tile_pool(name="relu", side="left", bufs=1)
    rrp = rr_cm.__enter__()
    rr = [rrp.tile([P, TQ], BF16, tag=f"r{k}", name=f"r{k}") for k in range(32)]
    # all 32 W2 row-chunks stay resident; their DMAs stream during FFN1
    # (this pool opens below rr so it carries no WAR deps on attention tiles)
    w2_cm = tc.tile_pool(name="w2s", side="left", bufs=1)
    w2p = w2_cm.__enter__()
    ws2 = []
    for k in range(32):
        w = w2p.tile([P, E], BF16, tag=f"w2_{k}", name=f"w2_{k}")
        nc.sync.dma_start(out=w[:], in_=d["w2"][k * P:(k + 1) * P, :])
        ws2.append(w)

    with tc.tile_pool(name="w1s", side="left", bufs=1) as w1p, \
         tc.tile_pool(name="ps_f1", bufs=4, space="PSUM") as fpp:
        for g in range(4):
            if g == 0:
                ws = ws_g0
            else:
                ws = []
                for k in range(8):
                    w = w1p.tile([P, 1024], BF16, tag=f"w1_{k}", bufs=2)
                    nc.sync.dma_start(
                        out=w[:], in_=d["w1"][k * P:(k + 1) * P,
                                              g * 1024:(g + 1) * 1024])
                    ws.append(w)
            for m in range(8):
                ps = fpp.tile([P, TQ], F32, tag="ps")
                for k in range(8):
                    nc.tensor.matmul(ps[:], ws[k][:, m * P:(m + 1) * P],
                                     h2T[k][:], start=(k == 0), stop=(k == 7))
                col = g * 8 + m
                nc.scalar.activation(rr[col][:], ps[:], AF.Relu,
                                     bias=bias["b1"][:, col:col + 1])
    # ---- FFN2, token-stationary, fused with final residual ----
    # lhsT = rr token-chunks (stationary), rhs = raw W2 row-chunks streamed
    # once; all four token chunks accumulate in parallel across the full
    # 8-bank psum (2 banks each), seeded with a K=1 bias broadcast. Drains
    # add the x2 residual and stream straight out token-major (no
    # transposes, contiguous output DMAs).
    with tc.tile_pool(name="outp", side="left", bufs=4) as outp, \
         tc.tile_pool(name="ps_f2", bufs=2, space="PSUM") as fpp:
        for i in range(4):
            ps2 = fpp.tile([P, 1024], F32, tag="pf", name=f"pf{i}")
            for nh in range(2):
                nc.tensor.matmul(ps2[:, nh * 512:nh * 512 + 512],
                                 ones_bf[0:1, :],
                                 b2row[:, nh * 512:nh * 512 + 512],
                                 start=True, stop=False, skip_group_check=True)
            for k in range(32):
                for nh in range(2):
                    nc.tensor.matmul(ps2[:, nh * 512:nh * 512 + 512],
                                     rr[k][:, i * P:(i + 1) * P],
                                     ws2[k][:, nh * 512:nh * 512 + 512],
                                     start=False, stop=(k == 31),
                                     skip_group_check=True)
            for nh in range(2):
                ot = outp.tile([P, TQ], F32, tag="o", name="o")
                nc.vector.scalar_tensor_tensor(
                    ot[:], ps2[:, nh * 512:nh * 512 + 512], 1.0,
                    x2[i][:, nh * 512:nh * 512 + 512], ALU.mult, ALU.add)
                nc.sync.dma_start(
                    out=out_d[i * P:(i + 1) * P, nh * 512:nh * 512 + 512],
                    in_=ot[:])

    w2_cm.__exit__(None, None, None)
    rr_cm.__exit__(None, None, None)
    x2s_cm.__exit__(None, None, None)
    w1g0_cm.__exit__(None, None, None)
    xt_cm.__exit__(None, None, None)
    const_cm.__exit__(None, None, None)


def _build():
    nc = bacc.Bacc("TRN2", target_bir_lowering=False, debug=False,
                num_devices=NCORES)
    d = {}

    def din(name, shape, dt=F32R):
        d[name] = nc.dram_tensor(name, shape, dt, kind="ExternalInput").ap()

    din("x", [TQ, E], F32)
    for n in ("wq", "wk", "wv"):
        din(n, [E, E], BF16)
    din("wp", [E, E], BF16)
    din("w1", [E, FF], BF16)
    din("w2", [FF, E], BF16)
    din("bprow", [1, E], BF16)
    din("b2row", [1, E], BF16)
    din("eye", [P, P], BF16)
    din("ones_bf", [P, P], BF16)
    for n, w in [("ln1g", 8), ("ln1b", 8), ("ln2g", 8), ("ln2b", 8),
                 ("bp", 8), ("b1", 32), ("b2", 8)]:
        din(n, [P, w], F32)
    out_d = nc.dram_tensor("out", [TQ, E], F32, kind="ExternalOutput").ap()
    for nm in ("cc_kinA", "cc_kinB", "cc_vinA", "cc_vinB"):
        d[nm] = nc.dram_tensor(nm, [4, P, TQ], BF16).ap()
    for nm in ("cc_koutA", "cc_koutB", "cc_voutA", "cc_voutB"):
        d[nm] = nc.dram_tensor(nm, [2, 4, P, TQ], BF16).ap()
    with nc.allow_low_precision(reason="fp32r compute"):
        with tile.TileContext(nc) as tc:
            _emit(nc, tc, d, out_d)
    nc.compile()
    return nc


def _get_nc():
    if "nc" not in _CACHE:
        _CACHE["nc"] = _build()
    return _CACHE["nc"]


def _colmajor_bias(v, width):
    return np.ascontiguousarray(np.asarray(v, np.float32).reshape(width, P).T)


def make_in_maps(x, ln1_g, ln1_b, Wq, Wk, Wv, Wp, bp, ln2_g, ln2_b,
                 W1, b1, W2, b2):
    x = np.asarray(x, dtype=np.float32)
    shared = {
        "wq": np.ascontiguousarray(
            np.transpose(np.asarray(Wq, np.float32), (1, 0, 2)).reshape(E, E)
        ).astype(ml_dtypes.bfloat16),
        "wk": np.ascontiguousarray(
            np.transpose(np.asarray(Wk, np.float32), (1, 0, 2)).reshape(E, E)
        ).astype(ml_dtypes.bfloat16),
        "wv": np.ascontiguousarray(
            np.transpose(np.asarray(Wv, np.float32), (1, 0, 2)).reshape(E, E)
        ).astype(ml_dtypes.bfloat16),
        "wp": np.asarray(Wp, np.float32).astype(ml_dtypes.bfloat16),
        "w1": np.asarray(W1, np.float32).astype(ml_dtypes.bfloat16),
        "w2": np.asarray(W2, np.float32).astype(ml_dtypes.bfloat16),
        "bprow": np.asarray(bp, np.float32).reshape(1, E).astype(
            ml_dtypes.bfloat16),
        "b2row": np.asarray(b2, np.float32).reshape(1, E).astype(
            ml_dtypes.bfloat16),
        "eye": np.eye(P, dtype=ml_dtypes.bfloat16),
        "ones_bf": np.ones((P, P), dtype=ml_dtypes.bfloat16),
        "ln1g": _colmajor_bias(ln1_g, 8),
        "ln1b": _colmajor_bias(ln1_b, 8),
        "ln2g": _colmajor_bias(ln2_g, 8),
        "ln2b": _colmajor_bias(ln2_b, 8),
        "bp": _colmajor_bias(bp, 8),
        "b1": _colmajor_bias(b1, 32),
        "b2": _colmajor_bias(b2, 8),
    }
    in_maps = []
    for c in range(NCORES):
        b = c // 2
        q0 = TQ * (c % 2)
        xb = x[b]
        x_roll = np.ascontiguousarray(np.concatenate([xb[q0:], xb[:q0]], axis=0)[:TQ])
        in_maps.append({"x": x_roll, **shared})
    return in_maps


def assemble_out(results):
    out = np.empty((B, T, E), dtype=np.float32)
    for c in range(NCORES):
        b = c // 2
        q0 = TQ * (c % 2)
        out[b, q0:q0 + TQ] = results[c]["out"]
    return out


def kernel(x, ln1_g, ln1_b, Wq, Wk, Wv, Wp, bp, ln2_g, ln2_b, W1, b1, W2, b2,
           **_ignored):
    in_maps = make_in_maps(x, ln1_g, ln1_b, Wq, Wk, Wv, Wp, bp,
                           ln2_g, ln2_b, W1, b1, W2, b2)
    nc = _get_nc()
    res = run_bass_kernel_spmd(nc, in_maps, core_ids=list(range(NCORES)))
    return assemble_out(res.results)
